# revision 5
# baseline (speedup 1.0000x reference)
"""Trainium2 Bass kernel: per-image segment-mean repaint (DeepgazeSpade).

Reference computation per image b:
  seg_ds        = segmap[::8, ::8]                  (nearest downsample: 384/48 = 512/64 = 8)
  sums[s, c]    = sum_{p : seg_ds[p] == s} feats[c, p]
  counts[s]     = |{p : seg_ds[p] == s}|
  avg[s, c]     = sums / max(counts, 1)             (0 for empty segments)
  out[c, y, x]  = avg[segmap[y, x], c]

Sharding: 8 cores = 4 images x 2 row-halves (pure data parallel, no
collectives). Each core computes the (cheap) per-image segment stats from the
full downsampled grid and paints its own half of the image.

Host prep (dtype casts / reshapes only): feats shipped pre-transposed and
pre-swizzled as bf16 [128, 24*257] (chunk-major, ones column fused at c=256
so counts fall out of the same matmul chain); downsampled seg ids shipped as
[128, 24] fp32 in chunk layout; paint seg ids shipped flat as bf16 (segment
ids < 128 are bf16-exact).

Per-core device algorithm (roofline: ~100.7 MB of fp32 output at ~360 GB/s
per-core DMA = ~280us; everything else hides behind the output stream):
  stats: 24 accumulating bf16 matmuls ohd[px,seg].T @ ft[px, 257] -> [seg,
         256 sums | counts] in fp32 PSUM; one-hot ohd built by
         tensor_scalar(is_equal) against an iota-row constant; avg = sums *
         reciprocal(max(counts,1)) written directly as bf16 (harness gate is
         2e-2 relative error; bf16 avg + bf16 feats is ~2e-3).
  paint: all 98304 seg ids stay resident in SBUF rows {0,32,64} (legal matmul
         operand base partitions). Per 1024-px pair: two K=1 bf16 matmuls
         broadcast the seg ids to all 128 partitions (fp32 PSUM);
         tensor_scalar(is_equal) against the partition-index iota gives the
         one-hot [s=128, 1024] in bf16 SBUF; two bf16 matmuls per 128-channel
         group (stationary avg, FWL) gather the channels into fp32 PSUM;
         scalar-engine copies evacuate to SBUF (DMA cannot touch PSUM); out
         DMAs alternate between the SP and Pool queues so issue overhead
         never gates the DMA engines. First/last pair run at 512-px
         granularity to shorten pipeline fill/drain.
"""

import numpy as np
import ml_dtypes

B, C = 4, 256
HF, WF = 48, 64
HIMG, WIMG = 384, 512
S = 128
NPIX_DS = HF * WF              # 3072 downsampled pixels
NCHUNK_DS = NPIX_DS // 128     # 24
CW = C + 1                     # 257: channels + fused ones column
HALF_ROWS = HIMG // 2          # 192
NPIX_HALF = HALF_ROWS * WIMG   # 98304 pixels per core
TILE = 512
PAIR = 2 * TILE                # 1024-px paint unit
NPAIRS = NPIX_HALF // PAIR     # 96
_CACHE = {}
LAST_RESULTS = None
TRACE = False


def _body(tc, out, ftT, ds, seg_bf, dummy=None):
    import concourse.mybir as mybir

    dt = mybir.dt
    eq = mybir.AluOpType.is_equal
    mul = mybir.AluOpType.mult
    nc = tc.nc

    with (
        tc.tile_pool(name="const", bufs=1) as cpool,
        tc.tile_pool(name="oh", bufs=4) as ohpool,
        tc.tile_pool(name="ob", bufs=6) as obpool,
    ):
        # ---- constants ----
        iota_row_i = cpool.tile([128, 128], dt.int32)
        nc.gpsimd.iota(iota_row_i[:], pattern=[[1, 128]], base=0, channel_multiplier=0)
        iota_row_f = cpool.tile([128, 128], dt.float32)
        nc.vector.tensor_copy(iota_row_f[:], iota_row_i[:])

        iota_col_i = cpool.tile([128, 1], dt.int32)
        nc.gpsimd.iota(iota_col_i[:], pattern=[[0, 1]], base=0, channel_multiplier=1)
        iota_col_f = cpool.tile([128, 1], dt.float32)
        nc.vector.tensor_copy(iota_col_f[:], iota_col_i[:])

        ones_bf = cpool.tile([128, 128], dt.bfloat16)
        nc.vector.memset(ones_bf[:], 1.0)

        # ---- loads (issue cadence is ~650ns/DMA, so few big DMAs win;
        # seg ids load last — paint consumes them later than stats needs ft)
        ds_f = cpool.tile([128, NCHUNK_DS], dt.float32)
        nc.sync.dma_start(ds_f[:], ds)

        # chunk j / partition p = ds pixel j*128 + p; free dim j*CW + c
        # (host ships this layout; 2 half-DMAs so early chunks land early)
        ft = cpool.tile([128, NCHUNK_DS * CW], dt.bfloat16)
        QW = NCHUNK_DS * CW // 2
        for q in range(2):
            nc.sync.dma_start(ft[:, q * QW:(q + 1) * QW],
                              ftT[:, q * QW:(q + 1) * QW])

        # all seg ids resident: rows {0,32,64} hold thirds of the half-image
        # (matmul operands may start at partition 0/32/64)
        SEGQ = NPIX_HALF // 3      # 32768
        seg_all = cpool.tile([128, SEGQ], dt.bfloat16)
        nc.sync.dma_start(
            seg_all[0:96:32, :],
            seg_bf.rearrange("(r f) -> r f", r=3),
        )

        # ---- stats: [seg, 256 sums | counts] via one accumulating chain ----
        stats_ctx = tc.tile_pool(name="ps", bufs=1, space="PSUM")
        ps = stats_ctx.__enter__()
        psum_s = ps.tile([128, CW], dt.float32)
        for j in range(NCHUNK_DS):
            ohd = ohpool.tile([128, 128], dt.bfloat16, tag="ohd")
            nc.vector.tensor_scalar(ohd[:], iota_row_f[:], ds_f[:, j:j + 1], None, eq)
            nc.tensor.matmul(
                psum_s[:], ohd[:], ft[:, j * CW:(j + 1) * CW],
                start=(j == 0), stop=(j == NCHUNK_DS - 1),
            )

        PPQ = SEGQ // PAIR         # 32 pairs per seg_all row

        def build_oh(pr):
            r = 32 * (pr // PPQ)
            o = (pr % PPQ) * PAIR
            oh = ohpool.tile([128, PAIR], dt.bfloat16, tag="oh")
            for half in range(2):
                hs = slice(half * TILE, (half + 1) * TILE)
                bc = bcpool.tile([128, TILE], dt.float32, tag="bc")
                nc.tensor.matmul(
                    bc[:], ones_bf[r:r + 1, :],
                    seg_all[r:r + 1, o + half * TILE:o + (half + 1) * TILE],
                    start=True, stop=True,
                )
                nc.vector.tensor_scalar(oh[:, hs], bc[:], iota_col_f[:], None, eq)
            return oh

        cnt1 = cpool.tile([128, 1], dt.float32)
        nc.vector.tensor_scalar_max(cnt1[:], psum_s[:, C:CW], 1.0)
        rec = cpool.tile([128, 1], dt.float32)
        nc.vector.reciprocal(rec[:], cnt1[:])
        avg_bf = cpool.tile([128, C], dt.bfloat16)
        nc.vector.tensor_scalar(avg_bf[:], psum_s[:, 0:C], rec[:], None, mul)
        stats_ctx.__exit__(None, None, None)

        # paint output is exactly the bf16 avg values (one-hot matmul gather
        # is exact in fp32 PSUM), so a bf16 output stream is bit-identical
        # after the host upcast — and halves the dominant out-DMA traffic.

        # ---- paint: 1024-px pairs ----
        bc_ctx = tc.tile_pool(name="bc", bufs=2, space="PSUM")
        bcpool = bc_ctx.__enter__()
        po_ctx = tc.tile_pool(name="po", bufs=3, space="PSUM")
        po = po_ctx.__enter__()
        for pr in range(NPAIRS):
            oh = build_oh(pr)
            # first/last pair at 512-px granularity: halves the pipeline
            # fill latency (first out-DMA) and the tail drain
            split = pr in (0, NPAIRS - 1)
            for cc in range(2):
                sl = slice(cc * 128, (cc + 1) * 128)
                eng = nc.sync if cc == 0 else nc.gpsimd
                cpy = nc.scalar.copy if cc == 0 else nc.vector.tensor_copy
                if split:
                    for half in range(2):
                        hs = slice(half * TILE, (half + 1) * TILE)
                        pot = po.tile([128, TILE], dt.float32, tag="po")
                        nc.tensor.matmul(pot[:], avg_bf[:, sl], oh[:, hs],
                                         start=True, stop=True)
                        ob = obpool.tile([128, TILE], dt.bfloat16, tag="ob")
                        cpy(ob[:], pot[:])
                        eng.dma_start(
                            out[sl, pr * PAIR + half * TILE:
                                pr * PAIR + (half + 1) * TILE], ob[:])
                else:
                    pot = po.tile([128, PAIR], dt.float32, tag="po")
                    for half in range(2):
                        hs = slice(half * TILE, (half + 1) * TILE)
                        nc.tensor.matmul(
                            pot[:, hs], avg_bf[:, sl], oh[:, hs],
                            start=True, stop=True,
                        )
                    ob = obpool.tile([128, PAIR], dt.bfloat16, tag="ob")
                    cpy(ob[:], pot[:])
                    eng.dma_start(
                        out[sl, pr * PAIR:(pr + 1) * PAIR], ob[:]
                    )
        po_ctx.__exit__(None, None, None)
        bc_ctx.__exit__(None, None, None)
        if dummy is not None:
            # bench mode: tiny ExternalOutput so the big `out` can be
            # internal DRAM (avoids shipping 100 MB/core through axon)
            nc.sync.dma_start(dummy.rearrange("(o f) -> o f", o=1),
                              ones_bf[0:1, 0:1])


def _build_nc(reps=1, bench=False):
    import concourse.bacc as bacc
    import concourse.mybir as mybir
    import concourse.tile as tile

    dt = mybir.dt
    nc = bacc.Bacc("TRN2", target_bir_lowering=False, debug=False,
                   enable_asserts=False)
    ftT = nc.dram_tensor("ftT", [128, NCHUNK_DS * CW], dt.bfloat16,
                         kind="ExternalInput").ap()
    ds = nc.dram_tensor("ds", [128, NCHUNK_DS], dt.float32,
                        kind="ExternalInput").ap()
    seg_bf = nc.dram_tensor("seg_bf", [NPIX_HALF], dt.bfloat16,
                            kind="ExternalInput").ap()
    if bench:
        out = nc.dram_tensor("out", [C, NPIX_HALF], dt.bfloat16).ap()
    else:
        out = nc.dram_tensor("out", [C, NPIX_HALF], dt.bfloat16,
                             kind="ExternalOutput").ap()
    dummy = None
    if bench:
        dummy = nc.dram_tensor("bench_out", [1], dt.bfloat16,
                               kind="ExternalOutput").ap()
    with tile.TileContext(nc) as tc:
        if reps == 1:
            _body(tc, out, ftT, ds, seg_bf, dummy)
        else:
            with tc.For_i(0, reps, 1):
                _body(tc, out, ftT, ds, seg_bf, dummy)
    nc.compile()
    return nc


def make_in_maps(F, seg):
    """F: [B, C, NPIX_DS] float32; seg: [B, HIMG, WIMG] int."""
    F = np.asarray(F, dtype=np.float32).reshape(B, C, NPIX_DS)
    seg = np.clip(np.asarray(seg), 0, S - 1).astype(np.int32)
    in_maps = []
    for core in range(8):
        b, h = core // 2, core % 2
        # ft[p, j*CW + c] = feats^T[j*128 + p, c], ones fused at c = C
        ftT = np.empty((NCHUNK_DS, 128, CW), dtype=ml_dtypes.bfloat16)
        ftT[:, :, :C] = F[b].T.reshape(NCHUNK_DS, 128, C)
        ftT[:, :, C] = 1.0
        ftT = np.ascontiguousarray(
            ftT.transpose(1, 0, 2).reshape(128, NCHUNK_DS * CW))
        dsb = seg[b, ::8, ::8].reshape(NCHUNK_DS, 128)
        seg_half = seg[b, h * HALF_ROWS:(h + 1) * HALF_ROWS, :].reshape(-1)
        in_maps.append({
            "ftT": ftT,
            "ds": np.ascontiguousarray(dsb.T.astype(np.float32)),
            "seg_bf": seg_half.astype(ml_dtypes.bfloat16),
        })
    return in_maps


def kernel(F_semantic_features, segmentation_mask, num_total_segments=None):
    global LAST_RESULTS
    from concourse.bass_utils import run_bass_kernel_spmd

    F = np.asarray(F_semantic_features, dtype=np.float32)
    seg = np.asarray(segmentation_mask)

    if "nc" not in _CACHE:
        _CACHE["nc"] = _build_nc()
    nc = _CACHE["nc"]

    in_maps = make_in_maps(F.reshape(B, C, NPIX_DS), seg)
    res = run_bass_kernel_spmd(nc, in_maps, core_ids=list(range(8)),
                               trace=bool(TRACE))
    LAST_RESULTS = res

    imgs = []
    for b in range(B):
        top = res.results[2 * b]["out"].reshape(C, HALF_ROWS, WIMG)
        bot = res.results[2 * b + 1]["out"].reshape(C, HALF_ROWS, WIMG)
        imgs.append(np.concatenate([top, bot], axis=1))
    # device ships bf16; upcast (dtype cast only) to the fp32 contract
    return np.stack(imgs).astype(np.float32)


if __name__ == "__main__":
    rng = np.random.default_rng(0)
    F = rng.standard_normal((B, C, HF, WF), dtype=np.float32)
    seg = rng.integers(0, S, size=(B, HIMG, WIMG)).astype(np.int64)
    outv = kernel(F, seg, S)
    print("out", outv.shape, outv.dtype, float(outv.mean()))



# revision 38
# speedup vs baseline: 1.0548x; 1.0548x over previous
"""Trainium2 Bass kernel: per-image segment-mean repaint (DeepgazeSpade).

Reference computation per image b:
  seg_ds        = segmap[::8, ::8]                  (nearest downsample: 384/48 = 512/64 = 8)
  sums[s, c]    = sum_{p : seg_ds[p] == s} feats[c, p]
  counts[s]     = |{p : seg_ds[p] == s}|
  avg[s, c]     = sums / max(counts, 1)             (0 for empty segments)
  out[c, y, x]  = avg[segmap[y, x], c]

Sharding: 8 cores = 4 images x 2 row-halves (pure data parallel, no
collectives). Each core computes the (cheap) per-image segment stats from the
full downsampled grid and paints its own half of the image.

Host prep (dtype casts / reshapes only): feats shipped pre-transposed and
pre-swizzled as bf16 [128, 24*257] (chunk-major, ones column fused at c=256
so counts fall out of the same matmul chain); downsampled seg ids shipped as
[128, 24] fp32 in chunk layout; paint seg ids shipped flat as bf16 (segment
ids < 128 are bf16-exact).

Per-core device algorithm (roofline: ~100.7 MB of fp32 output at ~360 GB/s
per-core DMA = ~280us; everything else hides behind the output stream):
  stats: 24 accumulating bf16 matmuls ohd[px,seg].T @ ft[px, 257] -> [seg,
         256 sums | counts] in fp32 PSUM; one-hot ohd built by
         tensor_scalar(is_equal) against an iota-row constant; avg = sums *
         reciprocal(max(counts,1)) written directly as bf16 (harness gate is
         2e-2 relative error; bf16 avg + bf16 feats is ~2e-3).
  paint: all 98304 seg ids stay resident in SBUF rows {0,32,64} (legal matmul
         operand base partitions). Per 1024-px pair: two K=1 bf16 matmuls
         broadcast the seg ids to all 128 partitions (fp32 PSUM);
         tensor_scalar(is_equal) against the partition-index iota gives the
         one-hot [s=128, 1024] in bf16 SBUF; two bf16 matmuls per 128-channel
         group (stationary avg, FWL) gather the channels into fp32 PSUM;
         scalar-engine copies evacuate to SBUF (DMA cannot touch PSUM); out
         DMAs alternate between the SP and Pool queues so issue overhead
         never gates the DMA engines. First/last pair run at 512-px
         granularity to shorten pipeline fill/drain.
"""

import numpy as np
import ml_dtypes

B, C = 4, 256
HF, WF = 48, 64
HIMG, WIMG = 384, 512
S = 128
NPIX_DS = HF * WF              # 3072 downsampled pixels
NCHUNK_DS = NPIX_DS // 128     # 24
CW = C + 1                     # 257: channels + fused ones column
HALF_ROWS = HIMG // 2          # 192
NPIX_HALF = HALF_ROWS * WIMG   # 98304 pixels per core
TILE = 512
PAIR = 2 * TILE                # 1024-px paint unit
NPAIRS = NPIX_HALF // PAIR     # 96
_CACHE = {}
LAST_RESULTS = None
TRACE = False


def _body(tc, out, ftT, ds, seg_bf, dummy=None):
    import concourse.mybir as mybir

    dt = mybir.dt
    eq = mybir.AluOpType.is_equal
    mul = mybir.AluOpType.mult
    nc = tc.nc

    with (
        tc.tile_pool(name="const", bufs=1) as cpool,
        tc.tile_pool(name="oh", bufs=4) as ohpool,
        tc.tile_pool(name="ob", bufs=6) as obpool,
    ):
        # ---- constants ----
        iota_row_i = cpool.tile([128, 128], dt.int32)
        nc.gpsimd.iota(iota_row_i[:], pattern=[[1, 128]], base=0, channel_multiplier=0)
        iota_row_f = cpool.tile([128, 128], dt.float32)
        nc.vector.tensor_copy(iota_row_f[:], iota_row_i[:])

        iota_col_i = cpool.tile([128, 1], dt.int32)
        nc.gpsimd.iota(iota_col_i[:], pattern=[[0, 1]], base=0, channel_multiplier=1)
        iota_col_f = cpool.tile([128, 1], dt.float32)
        nc.vector.tensor_copy(iota_col_f[:], iota_col_i[:])

        ones_bf = cpool.tile([128, 128], dt.bfloat16)
        nc.vector.memset(ones_bf[:], 1.0)

        # ---- loads: ds + seg first (pair-0 one-hot build needs seg; the
        # stats matmuls gate on the much larger ft anyway), then ft
        ds_f = cpool.tile([128, NCHUNK_DS], dt.float32)
        nc.sync.dma_start(ds_f[:], ds)

        # seg ids stream through partition-0-only tiles: partition_broadcast's
        # Q7 ucode has cpu 0 read the source, so the source MUST live on
        # partition 0 (the interp rejects any other start partition; HW would
        # silently read garbage). 8 chunks x 24KB with bufs=3 keeps partition
        # 0 usage at 72KB; chunk DMAs are issued two chunks ahead of use.
        NSEGCH = 8
        SEGCH = NPIX_HALF // NSEGCH      # 12288 px per chunk
        seg_r8 = seg_bf.rearrange("(r f) -> r f", r=NSEGCH)
        sp_ctx = tc.tile_pool(name="segp", bufs=3)
        segpool = sp_ctx.__enter__()
        seg_ch = {}

        def load_seg_chunk(c):
            t = segpool.tile([1, SEGCH], dt.bfloat16, tag="segp", name="segch")
            nc.sync.dma_start(t[:], seg_r8[c:c + 1, :])
            seg_ch[c] = t

        load_seg_chunk(0)
        load_seg_chunk(1)

        # chunk j / partition p = ds pixel j*128 + p; free dim j*CW + c
        # (host ships this layout; 2 half-DMAs so early chunks land early)
        ft = cpool.tile([128, NCHUNK_DS * CW], dt.bfloat16)
        QW = NCHUNK_DS * CW // 2
        for q in range(2):
            nc.sync.dma_start(ft[:, q * QW:(q + 1) * QW],
                              ftT[:, q * QW:(q + 1) * QW])

        # ---- stats: [seg, 256 sums | counts] via one accumulating chain ----
        stats_ctx = tc.tile_pool(name="ps", bufs=1, space="PSUM")
        ps = stats_ctx.__enter__()
        psum_s = ps.tile([128, CW], dt.float32)
        for j in range(NCHUNK_DS):
            # bufs=24: no slot reuse, so the scheduler can't create a chain
            # from these through a Pool-broadcast-gated paint one-hot
            ohd = ohpool.tile([128, 128], dt.bfloat16, tag="ohd", bufs=24)
            nc.vector.tensor_scalar(ohd[:], iota_row_f[:], ds_f[:, j:j + 1], None, eq)
            nc.tensor.matmul(
                psum_s[:], ohd[:], ft[:, j * CW:(j + 1) * CW],
                start=(j == 0), stop=(j == NCHUNK_DS - 1),
            )

        PPC = SEGCH // PAIR        # 12 pairs per seg chunk

        def build_oh(pr):
            # Broadcast 1024 seg ids to all 128 partitions on Pool
            # (SBUF->SBUF bf16, ~1.5us incl launch — small enough lumps that
            # nothing serializes behind them), then one-hot on DVE in 4x mode
            # (327ns/pair vs 1190ns reading fp32 PSUM). Every 8th pair rides
            # the PE K=1 matmul broadcast instead (seg chunks sit on
            # partition 0, a legal matmul operand base) to keep Pool well
            # under the DMA roofline.
            c, op_ = divmod(pr, PPC)
            if op_ == 0 and c + 2 < NSEGCH:
                load_seg_chunk(c + 2)
            o = op_ * PAIR
            oh = ohpool.tile([128, PAIR], dt.bfloat16, tag="oh", bufs=4)
            if pr % 8 == 0:
                for half in range(2):
                    hs = slice(half * TILE, (half + 1) * TILE)
                    bc = bcpool.tile([128, TILE], dt.float32, tag="bc")
                    nc.tensor.matmul(
                        bc[:], ones_bf[0:1, :],
                        seg_ch[c][0:1, o + half * TILE:o + (half + 1) * TILE],
                        start=True, stop=True,
                    )
                    nc.vector.tensor_scalar(oh[:, hs], bc[:], iota_col_f[:],
                                            None, eq)
            else:
                bcast = bspool.tile([128, PAIR], dt.bfloat16, tag="bcast")
                nc.gpsimd.partition_broadcast(
                    bcast[:], seg_ch[c][0:1, o:o + PAIR])
                nc.vector.tensor_scalar(oh[:], bcast[:], iota_col_f[:],
                                        None, eq)
            return oh

        cnt1 = cpool.tile([128, 1], dt.float32)
        nc.vector.tensor_scalar_max(cnt1[:], psum_s[:, C:CW], 1.0)
        rec = cpool.tile([128, 1], dt.float32)
        nc.vector.reciprocal(rec[:], cnt1[:])
        avg_bf = cpool.tile([128, C], dt.bfloat16)
        nc.vector.tensor_scalar(avg_bf[:], psum_s[:, 0:C], rec[:], None, mul)
        stats_ctx.__exit__(None, None, None)

        # paint output is exactly the bf16 avg values (one-hot matmul gather
        # is exact in fp32 PSUM), so a bf16 output stream is bit-identical
        # after the host upcast — and halves the dominant out-DMA traffic.

        # ---- paint: 1024-px pairs, software-pipelined one pair ahead ----
        # PE executes in order, so the broadcast matmuls for pair p+1 are
        # issued BEFORE the gathers of pair p: DVE's is_equal for p+1 then
        # overlaps the gathers of p instead of serializing a PE<->DVE round
        # trip per pair. Engine budget (TimelineSim): DVE one-hot ~130us, Act
        # evac ~112us, Pool evac ~127us, PE ~126us, DMA engines ~145us (the
        # target bound). All out-DMAs ride SP's hardware DGE (565ns SEQ /
        # 625ns HWDGE per DMA; Pool-issued SWDGE DMAs would eat 994ns of
        # Pool ENGINE time each).
        bs_ctx = tc.tile_pool(name="bs", bufs=3)
        bspool = bs_ctx.__enter__()
        bc_ctx = tc.tile_pool(name="bc", bufs=2, space="PSUM")
        bcpool = bc_ctx.__enter__()
        po_ctx = tc.tile_pool(name="po", bufs=3, space="PSUM")
        po = po_ctx.__enter__()
        # software-pipelined one pair ahead: PE executes in order, so the
        # (rare) PE broadcast for pair p+1 is issued BEFORE the gathers of
        # pair p, and DVE's is_equal for p+1 overlaps the gathers of p
        oh_cur = build_oh(0)
        for pr in range(NPAIRS):
            oh_next = build_oh(pr + 1) if pr + 1 < NPAIRS else None
            for cc in range(2):
                sl = slice(cc * 128, (cc + 1) * 128)
                pot = po.tile([128, PAIR], dt.float32, tag="po")
                for half in range(2):
                    hs = slice(half * TILE, (half + 1) * TILE)
                    nc.tensor.matmul(
                        pot[:, hs], avg_bf[:, sl], oh_cur[:, hs],
                        start=True, stop=True,
                    )
                # evac: GPSIMD can't touch PSUM, so only Act (1038ns) and
                # DVE (1191ns) can drain it; Act takes all of cc0 plus a
                # third of cc1 (~128us each incl. DVE's one-hot work)
                if cc == 0 or pr % 3 == 0:
                    cpy = nc.scalar.copy
                else:
                    cpy = nc.vector.tensor_copy
                ob = obpool.tile([128, PAIR], dt.bfloat16, tag="ob")
                cpy(ob[:], pot[:])
                nc.sync.dma_start(out[sl, pr * PAIR:(pr + 1) * PAIR], ob[:])
            oh_cur = oh_next
        po_ctx.__exit__(None, None, None)
        bc_ctx.__exit__(None, None, None)
        bs_ctx.__exit__(None, None, None)
        sp_ctx.__exit__(None, None, None)
        if dummy is not None:
            # bench mode: tiny ExternalOutput so the big `out` can be
            # internal DRAM (avoids shipping 100 MB/core through axon)
            nc.sync.dma_start(dummy.rearrange("(o f) -> o f", o=1),
                              ones_bf[0:1, 0:1])


def _build_nc(reps=1, bench=False):
    import concourse.bacc as bacc
    import concourse.mybir as mybir
    import concourse.tile as tile

    dt = mybir.dt
    nc = bacc.Bacc("TRN2", target_bir_lowering=False, debug=False,
                   enable_asserts=False)
    ftT = nc.dram_tensor("ftT", [128, NCHUNK_DS * CW], dt.bfloat16,
                         kind="ExternalInput").ap()
    ds = nc.dram_tensor("ds", [128, NCHUNK_DS], dt.float32,
                        kind="ExternalInput").ap()
    seg_bf = nc.dram_tensor("seg_bf", [NPIX_HALF], dt.bfloat16,
                            kind="ExternalInput").ap()
    if bench:
        out = nc.dram_tensor("out", [C, NPIX_HALF], dt.bfloat16).ap()
    else:
        out = nc.dram_tensor("out", [C, NPIX_HALF], dt.bfloat16,
                             kind="ExternalOutput").ap()
    dummy = None
    if bench:
        dummy = nc.dram_tensor("bench_out", [1], dt.bfloat16,
                               kind="ExternalOutput").ap()
    with tile.TileContext(nc) as tc:
        if reps == 1:
            _body(tc, out, ftT, ds, seg_bf, dummy)
        else:
            with tc.For_i(0, reps, 1):
                _body(tc, out, ftT, ds, seg_bf, dummy)
    nc.compile()
    return nc


def make_in_maps(F, seg):
    """F: [B, C, NPIX_DS] float32; seg: [B, HIMG, WIMG] int."""
    F = np.asarray(F, dtype=np.float32).reshape(B, C, NPIX_DS)
    seg = np.clip(np.asarray(seg), 0, S - 1).astype(np.int32)
    in_maps = []
    for core in range(8):
        b, h = core // 2, core % 2
        # ft[p, j*CW + c] = feats^T[j*128 + p, c], ones fused at c = C
        ftT = np.empty((NCHUNK_DS, 128, CW), dtype=ml_dtypes.bfloat16)
        ftT[:, :, :C] = F[b].T.reshape(NCHUNK_DS, 128, C)
        ftT[:, :, C] = 1.0
        ftT = np.ascontiguousarray(
            ftT.transpose(1, 0, 2).reshape(128, NCHUNK_DS * CW))
        dsb = seg[b, ::8, ::8].reshape(NCHUNK_DS, 128)
        seg_half = seg[b, h * HALF_ROWS:(h + 1) * HALF_ROWS, :].reshape(-1)
        in_maps.append({
            "ftT": ftT,
            "ds": np.ascontiguousarray(dsb.T.astype(np.float32)),
            "seg_bf": seg_half.astype(ml_dtypes.bfloat16),
        })
    return in_maps


def kernel(F_semantic_features, segmentation_mask, num_total_segments=None):
    global LAST_RESULTS
    from concourse.bass_utils import run_bass_kernel_spmd

    F = np.asarray(F_semantic_features, dtype=np.float32)
    seg = np.asarray(segmentation_mask)

    if "nc" not in _CACHE:
        _CACHE["nc"] = _build_nc()
    nc = _CACHE["nc"]

    in_maps = make_in_maps(F.reshape(B, C, NPIX_DS), seg)
    res = run_bass_kernel_spmd(nc, in_maps, core_ids=list(range(8)),
                               trace=bool(TRACE))
    LAST_RESULTS = res

    imgs = []
    for b in range(B):
        top = res.results[2 * b]["out"].reshape(C, HALF_ROWS, WIMG)
        bot = res.results[2 * b + 1]["out"].reshape(C, HALF_ROWS, WIMG)
        imgs.append(np.concatenate([top, bot], axis=1))
    # device ships bf16; upcast (dtype cast only) to the fp32 contract
    return np.stack(imgs).astype(np.float32)


if __name__ == "__main__":
    rng = np.random.default_rng(0)
    F = rng.standard_normal((B, C, HF, WF), dtype=np.float32)
    seg = rng.integers(0, S, size=(B, HIMG, WIMG)).astype(np.int64)
    outv = kernel(F, seg, S)
    print("out", outv.shape, outv.dtype, float(outv.mean()))



# revision 41
# speedup vs baseline: 1.4823x; 1.4053x over previous
"""Trainium2 Bass kernel: per-image segment-mean repaint (DeepgazeSpade).

Reference computation per image b:
  seg_ds        = segmap[::8, ::8]                  (nearest downsample: 384/48 = 512/64 = 8)
  sums[s, c]    = sum_{p : seg_ds[p] == s} feats[c, p]
  counts[s]     = |{p : seg_ds[p] == s}|
  avg[s, c]     = sums / max(counts, 1)             (0 for empty segments)
  out[c, y, x]  = avg[segmap[y, x], c]

Sharding: 8 cores = 4 images x 2 row-halves (pure data parallel, no
collectives). Each core computes the (cheap) per-image segment stats from the
full downsampled grid and paints its own half of the image.

Host prep (dtype casts / reshapes only): feats shipped pre-transposed and
pre-swizzled as bf16 [128, 24*257] (chunk-major, ones column fused at c=256
so counts fall out of the same matmul chain); downsampled seg ids shipped as
[128, 24] fp32 in chunk layout; paint seg ids shipped flat as bf16 (segment
ids < 128 are bf16-exact).

Per-core device algorithm (roofline: ~100.7 MB of fp32 output at ~360 GB/s
per-core DMA = ~280us; everything else hides behind the output stream):
  stats: 24 accumulating bf16 matmuls ohd[px,seg].T @ ft[px, 257] -> [seg,
         256 sums | counts] in fp32 PSUM; one-hot ohd built by
         tensor_scalar(is_equal) against an iota-row constant; avg = sums *
         reciprocal(max(counts,1)) written directly as bf16 (harness gate is
         2e-2 relative error; bf16 avg + bf16 feats is ~2e-3).
  paint: all 98304 seg ids stay resident in SBUF rows {0,32,64} (legal matmul
         operand base partitions). Per 1024-px pair: two K=1 bf16 matmuls
         broadcast the seg ids to all 128 partitions (fp32 PSUM);
         tensor_scalar(is_equal) against the partition-index iota gives the
         one-hot [s=128, 1024] in bf16 SBUF; two bf16 matmuls per 128-channel
         group (stationary avg, FWL) gather the channels into fp32 PSUM;
         scalar-engine copies evacuate to SBUF (DMA cannot touch PSUM); out
         DMAs alternate between the SP and Pool queues so issue overhead
         never gates the DMA engines. First/last pair run at 512-px
         granularity to shorten pipeline fill/drain.
"""

import numpy as np
import ml_dtypes

B, C = 4, 256
HF, WF = 48, 64
HIMG, WIMG = 384, 512
S = 128
NPIX_DS = HF * WF              # 3072 downsampled pixels
NCHUNK_DS = NPIX_DS // 128     # 24
CW = C + 1                     # 257: channels + fused ones column
HALF_ROWS = HIMG // 2          # 192
NPIX_HALF = HALF_ROWS * WIMG   # 98304 pixels per core
TILE = 512
PAIR = 2 * TILE                # 1024-px paint unit
NPAIRS = NPIX_HALF // PAIR     # 96
_CACHE = {}
LAST_RESULTS = None
TRACE = False
# tuning knobs (build-time): pair pr uses the PE K=1 matmul broadcast when
# pr % PE_BC_EVERY == 0, else Pool partition_broadcast. cc1 evac goes to DVE
# per EVAC_PAT (0: pr%3!=0 -> DVE [64 pairs]; 1: pr%8<3 -> DVE [36 pairs]).
PE_BC_EVERY = 8
EVAC_PAT = 0


def _body(tc, out, ftT, ds, seg_bf, dummy=None):
    import concourse.mybir as mybir

    dt = mybir.dt
    eq = mybir.AluOpType.is_equal
    mul = mybir.AluOpType.mult
    nc = tc.nc

    with (
        tc.tile_pool(name="const", bufs=1) as cpool,
        tc.tile_pool(name="oh", bufs=4) as ohpool,
        tc.tile_pool(name="ob", bufs=6) as obpool,
    ):
        # ---- constants ----
        iota_row_i = cpool.tile([128, 128], dt.int32)
        nc.gpsimd.iota(iota_row_i[:], pattern=[[1, 128]], base=0, channel_multiplier=0)
        iota_row_f = cpool.tile([128, 128], dt.float32)
        nc.vector.tensor_copy(iota_row_f[:], iota_row_i[:])

        iota_col_i = cpool.tile([128, 1], dt.int32)
        nc.gpsimd.iota(iota_col_i[:], pattern=[[0, 1]], base=0, channel_multiplier=1)
        iota_col_f = cpool.tile([128, 1], dt.float32)
        nc.vector.tensor_copy(iota_col_f[:], iota_col_i[:])

        ones_bf = cpool.tile([128, 128], dt.bfloat16)
        nc.vector.memset(ones_bf[:], 1.0)

        # ---- loads: ds + seg first (pair-0 one-hot build needs seg; the
        # stats matmuls gate on the much larger ft anyway), then ft
        ds_f = cpool.tile([128, NCHUNK_DS], dt.float32)
        nc.sync.dma_start(ds_f[:], ds)

        # seg ids stream through partition-0-only tiles: partition_broadcast's
        # Q7 ucode has cpu 0 read the source, so the source MUST live on
        # partition 0 (the interp rejects any other start partition; HW would
        # silently read garbage). 8 chunks x 24KB with bufs=3 keeps partition
        # 0 usage at 72KB; chunk DMAs are issued two chunks ahead of use.
        NSEGCH = 8
        SEGCH = NPIX_HALF // NSEGCH      # 12288 px per chunk
        seg_r8 = seg_bf.rearrange("(r f) -> r f", r=NSEGCH)
        sp_ctx = tc.tile_pool(name="segp", bufs=3)
        segpool = sp_ctx.__enter__()
        seg_ch = {}

        def load_seg_chunk(c):
            t = segpool.tile([1, SEGCH], dt.bfloat16, tag="segp", name="segch")
            nc.sync.dma_start(t[:], seg_r8[c:c + 1, :])
            seg_ch[c] = t

        load_seg_chunk(0)
        load_seg_chunk(1)

        # chunk j / partition p = ds pixel j*128 + p; free dim j*CW + c
        # (host ships this layout; 2 half-DMAs so early chunks land early)
        ft = cpool.tile([128, NCHUNK_DS * CW], dt.bfloat16)
        QW = NCHUNK_DS * CW // 2
        for q in range(2):
            nc.sync.dma_start(ft[:, q * QW:(q + 1) * QW],
                              ftT[:, q * QW:(q + 1) * QW])

        # ---- stats: [seg, 256 sums | counts] via one accumulating chain ----
        stats_ctx = tc.tile_pool(name="ps", bufs=1, space="PSUM")
        ps = stats_ctx.__enter__()
        psum_s = ps.tile([128, CW], dt.float32)
        for j in range(NCHUNK_DS):
            # bufs=24: no slot reuse, so the scheduler can't create a chain
            # from these through a Pool-broadcast-gated paint one-hot
            ohd = ohpool.tile([128, 128], dt.bfloat16, tag="ohd", bufs=24)
            nc.vector.tensor_scalar(ohd[:], iota_row_f[:], ds_f[:, j:j + 1], None, eq)
            nc.tensor.matmul(
                psum_s[:], ohd[:], ft[:, j * CW:(j + 1) * CW],
                start=(j == 0), stop=(j == NCHUNK_DS - 1),
            )

        PPC = SEGCH // PAIR        # 12 pairs per seg chunk

        def build_oh(pr):
            # Broadcast 1024 seg ids to all 128 partitions on Pool
            # (SBUF->SBUF bf16, ~1.5us incl launch — small enough lumps that
            # nothing serializes behind them), then one-hot on DVE in 4x mode
            # (327ns/pair vs 1190ns reading fp32 PSUM). Every 8th pair rides
            # the PE K=1 matmul broadcast instead (seg chunks sit on
            # partition 0, a legal matmul operand base) to keep Pool well
            # under the DMA roofline.
            c, op_ = divmod(pr, PPC)
            if op_ == 0 and c + 2 < NSEGCH:
                load_seg_chunk(c + 2)
            o = op_ * PAIR
            oh = ohpool.tile([128, PAIR], dt.bfloat16, tag="oh", bufs=4)
            if pr % PE_BC_EVERY == 0:
                for half in range(2):
                    hs = slice(half * TILE, (half + 1) * TILE)
                    bc = bcpool.tile([128, TILE], dt.float32, tag="bc")
                    nc.tensor.matmul(
                        bc[:], ones_bf[0:1, :],
                        seg_ch[c][0:1, o + half * TILE:o + (half + 1) * TILE],
                        start=True, stop=True,
                    )
                    nc.vector.tensor_scalar(oh[:, hs], bc[:], iota_col_f[:],
                                            None, eq)
            else:
                bcast = bspool.tile([128, PAIR], dt.bfloat16, tag="bcast")
                nc.gpsimd.partition_broadcast(
                    bcast[:], seg_ch[c][0:1, o:o + PAIR])
                nc.vector.tensor_scalar(oh[:], bcast[:], iota_col_f[:],
                                        None, eq)
            return oh

        cnt1 = cpool.tile([128, 1], dt.float32)
        nc.vector.tensor_scalar_max(cnt1[:], psum_s[:, C:CW], 1.0)
        rec = cpool.tile([128, 1], dt.float32)
        nc.vector.reciprocal(rec[:], cnt1[:])
        avg_bf = cpool.tile([128, C], dt.bfloat16)
        nc.vector.tensor_scalar(avg_bf[:], psum_s[:, 0:C], rec[:], None, mul)
        stats_ctx.__exit__(None, None, None)

        # paint output is exactly the bf16 avg values (one-hot matmul gather
        # is exact in fp32 PSUM), so a bf16 output stream is bit-identical
        # after the host upcast — and halves the dominant out-DMA traffic.

        # ---- paint: 1024-px pairs, software-pipelined one pair ahead ----
        # PE executes in order, so the broadcast matmuls for pair p+1 are
        # issued BEFORE the gathers of pair p: DVE's is_equal for p+1 then
        # overlaps the gathers of p instead of serializing a PE<->DVE round
        # trip per pair. Engine budget (TimelineSim): DVE one-hot ~130us, Act
        # evac ~112us, Pool evac ~127us, PE ~126us, DMA engines ~145us (the
        # target bound). All out-DMAs ride SP's hardware DGE (565ns SEQ /
        # 625ns HWDGE per DMA; Pool-issued SWDGE DMAs would eat 994ns of
        # Pool ENGINE time each).
        bs_ctx = tc.tile_pool(name="bs", bufs=3)
        bspool = bs_ctx.__enter__()
        bc_ctx = tc.tile_pool(name="bc", bufs=2, space="PSUM")
        bcpool = bc_ctx.__enter__()
        po_ctx = tc.tile_pool(name="po", bufs=3, space="PSUM")
        po = po_ctx.__enter__()
        # software-pipelined one pair ahead: PE executes in order, so the
        # (rare) PE broadcast for pair p+1 is issued BEFORE the gathers of
        # pair p, and DVE's is_equal for p+1 overlaps the gathers of p
        oh_cur = build_oh(0)
        for pr in range(NPAIRS):
            oh_next = build_oh(pr + 1) if pr + 1 < NPAIRS else None
            for cc in range(2):
                sl = slice(cc * 128, (cc + 1) * 128)
                pot = po.tile([128, PAIR], dt.float32, tag="po")
                for half in range(2):
                    hs = slice(half * TILE, (half + 1) * TILE)
                    nc.tensor.matmul(
                        pot[:, hs], avg_bf[:, sl], oh_cur[:, hs],
                        start=True, stop=True,
                    )
                # evac: GPSIMD can't touch PSUM, so only Act (1038ns) and
                # DVE (1191ns) can drain it; Act takes all of cc0 plus a
                # third of cc1 (~128us each incl. DVE's one-hot work)
                if EVAC_PAT == 0:
                    to_dve = cc == 1 and pr % 3 != 0
                else:
                    to_dve = cc == 1 and pr % 8 < 3
                cpy = nc.vector.tensor_copy if to_dve else nc.scalar.copy
                ob = obpool.tile([128, PAIR], dt.bfloat16, tag="ob")
                cpy(ob[:], pot[:])
                nc.sync.dma_start(out[sl, pr * PAIR:(pr + 1) * PAIR], ob[:])
            oh_cur = oh_next
        po_ctx.__exit__(None, None, None)
        bc_ctx.__exit__(None, None, None)
        bs_ctx.__exit__(None, None, None)
        sp_ctx.__exit__(None, None, None)
        if dummy is not None:
            # bench mode: tiny ExternalOutput so the big `out` can be
            # internal DRAM (avoids shipping 100 MB/core through axon)
            nc.sync.dma_start(dummy.rearrange("(o f) -> o f", o=1),
                              ones_bf[0:1, 0:1])


def _build_nc(reps=1, bench=False):
    import concourse.bacc as bacc
    import concourse.mybir as mybir
    import concourse.tile as tile

    dt = mybir.dt
    nc = bacc.Bacc("TRN2", target_bir_lowering=False, debug=False,
                   enable_asserts=False)
    ftT = nc.dram_tensor("ftT", [128, NCHUNK_DS * CW], dt.bfloat16,
                         kind="ExternalInput").ap()
    ds = nc.dram_tensor("ds", [128, NCHUNK_DS], dt.float32,
                        kind="ExternalInput").ap()
    seg_bf = nc.dram_tensor("seg_bf", [NPIX_HALF], dt.bfloat16,
                            kind="ExternalInput").ap()
    if bench:
        out = nc.dram_tensor("out", [C, NPIX_HALF], dt.bfloat16).ap()
    else:
        out = nc.dram_tensor("out", [C, NPIX_HALF], dt.bfloat16,
                             kind="ExternalOutput").ap()
    dummy = None
    if bench:
        dummy = nc.dram_tensor("bench_out", [1], dt.bfloat16,
                               kind="ExternalOutput").ap()
    with tile.TileContext(nc) as tc:
        if reps == 1:
            _body(tc, out, ftT, ds, seg_bf, dummy)
        else:
            with tc.For_i(0, reps, 1):
                _body(tc, out, ftT, ds, seg_bf, dummy)
    nc.compile()
    return nc


def make_in_maps(F, seg):
    """F: [B, C, NPIX_DS] float32; seg: [B, HIMG, WIMG] int."""
    F = np.asarray(F, dtype=np.float32).reshape(B, C, NPIX_DS)
    seg = np.clip(np.asarray(seg), 0, S - 1).astype(np.int32)
    in_maps = []
    for core in range(8):
        b, h = core // 2, core % 2
        # ft[p, j*CW + c] = feats^T[j*128 + p, c], ones fused at c = C
        ftT = np.empty((NCHUNK_DS, 128, CW), dtype=ml_dtypes.bfloat16)
        ftT[:, :, :C] = F[b].T.reshape(NCHUNK_DS, 128, C)
        ftT[:, :, C] = 1.0
        ftT = np.ascontiguousarray(
            ftT.transpose(1, 0, 2).reshape(128, NCHUNK_DS * CW))
        dsb = seg[b, ::8, ::8].reshape(NCHUNK_DS, 128)
        seg_half = seg[b, h * HALF_ROWS:(h + 1) * HALF_ROWS, :].reshape(-1)
        in_maps.append({
            "ftT": ftT,
            "ds": np.ascontiguousarray(dsb.T.astype(np.float32)),
            "seg_bf": seg_half.astype(ml_dtypes.bfloat16),
        })
    return in_maps


def kernel(F_semantic_features, segmentation_mask, num_total_segments=None):
    global LAST_RESULTS
    from concourse.bass_utils import run_bass_kernel_spmd

    F = np.asarray(F_semantic_features, dtype=np.float32)
    seg = np.asarray(segmentation_mask)

    if "nc" not in _CACHE:
        _CACHE["nc"] = _build_nc()
    nc = _CACHE["nc"]

    in_maps = make_in_maps(F.reshape(B, C, NPIX_DS), seg)
    res = run_bass_kernel_spmd(nc, in_maps, core_ids=list(range(8)),
                               trace=bool(TRACE))
    LAST_RESULTS = res

    imgs = []
    for b in range(B):
        top = res.results[2 * b]["out"].reshape(C, HALF_ROWS, WIMG)
        bot = res.results[2 * b + 1]["out"].reshape(C, HALF_ROWS, WIMG)
        imgs.append(np.concatenate([top, bot], axis=1))
    # device ships bf16; upcast (dtype cast only) to the fp32 contract
    return np.stack(imgs).astype(np.float32)


if __name__ == "__main__":
    rng = np.random.default_rng(0)
    F = rng.standard_normal((B, C, HF, WF), dtype=np.float32)
    seg = rng.integers(0, S, size=(B, HIMG, WIMG)).astype(np.int64)
    outv = kernel(F, seg, S)
    print("out", outv.shape, outv.dtype, float(outv.mean()))

